# revision 29
# baseline (speedup 1.0000x reference)
"""Distributed mean-squared-distance kernel for Trainium2 (8 NeuronCores).

Computes  out[b] = mean_n ||x[b] - features[n]||^2  for x:[1024,128],
features:[100000,128].

Because the mean is linear, the full [B, N] distance matrix is never needed:

    out[b] = ||x_b||^2 + (1/N) * sum_n ||f_n||^2 - (2/N) * x_b . (sum_n f_n)

Each core streams a 1/8 shard of `features` once (memory-bound roofline:
~6.55 MB/core at ~350 GB/s).  The shard is cast fp32->bf16 inside the DMA
(SWDGE inline cast): HBM traffic is unchanged, but every downstream engine
runs at its 16-bit fast path.  Precision loss is ~1e-5 relative -- the
noise-sensitive |x|^2 term stays fp32.

Engine split, all overlapped with the DMA stream:

  * PE: S1 = sum_n f_n via an all-ones bf16 stationary matrix -- the
    ones-matmul output is the column sum replicated across all 128 output
    partitions, so no cross-partition reduce is ever needed.  26 bf16
    matmuls (free dim 512) accumulate into one PSUM bank.  A final
    ones/N matmul likewise turns the per-partition sum-of-squares column
    into a replicated S2/N scalar.
  * DVE: per-tile square+reduce in bf16 (2 elem/cycle), the fp32 x-path,
    and the final combine.
  * ACT / GPSIMD-compute: unused (per-instruction overhead).

The host gather step sums the 8 partial outputs (the all-reduce of the
sharding hint).
"""

import sys

sys.path.insert(0, "/opt/trn_rl_repo")

import numpy as np

import concourse.bacc as bacc
import concourse.tile as tile
from concourse import mybir
from concourse import bass_utils

P = 128                    # SBUF partitions
B, D, N = 1024, 128, 100000
NCORES = 8
TPP = 100                  # feature rows per partition per core
RPC = P * TPP              # 12800 feature rows per core (padded shard)
PAD_N = RPC * NCORES       # 102400 rows after zero-padding
BT = B // P                # 8 x-rows per partition
# Feature DMA tiles (rows-per-partition each).  The first two go over HWDGE
# as fp32 (the Sync engine is ready ~2us before the SWDGE Q7 path, so this
# fills the otherwise half-idle head of the stream); the rest are SWDGE
# casting DMAs.  The small tile goes last so the post-stream dependency
# chain (last tile's square + matmuls + completion-receipt lag) is short.
HW_CHUNKS = [8, 8]
SW_CHUNKS = [8] * 10 + [4]
CHUNKS = HW_CHUNKS + SW_CHUNKS
MMF = 512                  # matmul moving free size (one PSUM bank of fp32)

F32 = mybir.dt.float32
BF16 = mybir.dt.bfloat16
AX = mybir.AxisListType
OP = mybir.AluOpType


def _build():
    nc = bacc.Bacc("TRN2", debug=False, num_devices=NCORES)
    f_d = nc.dram_tensor("features", [RPC, D], F32, kind="ExternalInput").ap()
    x_d = nc.dram_tensor("x", [B, D], F32, kind="ExternalInput").ap()
    y_d = nc.dram_tensor("y", [P, BT], F32, kind="ExternalOutput").ap()

    # Row r of the shard maps to partition r // TPP, chunk r % TPP: each
    # partition reads one contiguous (TPP*D*4 B) run of DRAM per core.
    f_view = f_d.rearrange("(p t) d -> p t d", p=P)    # [128, 100, 128]
    x_view = x_d.rearrange("(p t) d -> p t d", p=P)    # [128, 8, 128]

    with tile.TileContext(nc) as tc:
        with (
            tc.tile_pool(name="fpool", bufs=1) as fpool,
            tc.tile_pool(name="scratch", bufs=1) as scratch,
            tc.tile_pool(name="small", bufs=1) as small,
            tc.tile_pool(name="psum", bufs=1, space="PSUM") as psum,
        ):
            # x via HWDGE (no cast; the x-path stays fp32).
            xt = small.tile([P, BT, D], F32)
            nc.sync.dma_start(out=xt, in_=x_view)

            ones = small.tile([P, P], BF16)
            nc.vector.memset(ones, 1.0)
            onesf = small.tile([P, P], F32)
            nc.vector.memset(onesf, 1.0)

            # The PE boots throttled (HAM clock gate, 1.2 GHz) and only
            # reaches 2.4 GHz after ~3.4us of sustained activity.  Keep it
            # busy with junk matmuls through the otherwise-dead window before
            # the first feature tile lands, so the real matmuls run warm.
            warmp = psum.tile([P, P], F32)
            for w in range(60):
                nc.tensor.matmul(
                    warmp, lhsT=ones, rhs=ones, start=True, stop=True,
                    skip_group_check=True,
                )

            # Feature stream: per tile one casting DMA, one DVE bf16 square,
            # and accumulating PE ones-matmuls for BOTH reductions -- S1 from
            # the raw tile, and the squared tile's column sums (toward S2)
            # into a second PSUM bank.  DVE never reduces the stream.
            s1p = psum.tile([P, MMF], F32)
            sqp = psum.tile([P, MMF], F32)
            nmm_total = sum(-(-sz * D // MMF) for sz in CHUNKS)
            sqscr = [
                scratch.tile([P, CHUNKS[0] * D], BF16, name=f"sqscr{k}")
                for k in range(4)
            ]
            sqscrf = [
                scratch.tile([P, CHUNKS[0] * D], F32, name=f"sqscrf{k}")
                for k in range(len(HW_CHUNKS))
            ]
            mm_idx = 0
            off = 0
            for i, sz in enumerate(CHUNKS):
                hw = i < len(HW_CHUNKS)
                dt_t = F32 if hw else BF16
                ft = fpool.tile([P, sz, D], dt_t, tag=f"ft{i}")
                if hw:
                    nc.sync.dma_start(out=ft, in_=f_view[:, off : off + sz, :])
                    scr = sqscrf[i]
                    lhs = onesf
                else:
                    nc.gpsimd.dma_start(
                        out=ft, in_=f_view[:, off : off + sz, :]
                    )
                    scr = sqscr[i % 4]
                    lhs = ones
                flat = ft.rearrange("p t d -> p (t d)")
                nc.vector.tensor_mul(out=scr[:, : sz * D], in0=flat, in1=flat)
                for j0 in range(0, sz * D, MMF):
                    j1 = min(j0 + MMF, sz * D)
                    first = mm_idx == 0
                    last = mm_idx == nmm_total - 1
                    nc.tensor.matmul(
                        s1p[:, : j1 - j0],
                        lhsT=lhs,
                        rhs=flat[:, j0:j1],
                        start=first,
                        stop=last,
                        skip_group_check=True,
                    )
                    nc.tensor.matmul(
                        sqp[:, : j1 - j0],
                        lhsT=lhs,
                        rhs=scr[:, j0:j1],
                        start=first,
                        stop=last,
                        skip_group_check=True,
                    )
                    mm_idx += 1
                off += sz
                if i == 5:
                    # x2 path, emitted mid-stream: x has arrived by now, and
                    # putting it first would park unready instructions at the
                    # DVE queue head (wait-queue depth 4) and stall the tile
                    # squares behind them.
                    xx = scratch.tile([P, BT, D], F32)
                    nc.vector.tensor_mul(out=xx, in0=xt, in1=xt)
                    x2cols = small.tile([P, BT], F32)
                    nc.vector.tensor_reduce(
                        out=x2cols, in_=xx, axis=AX.X, op=OP.add
                    )
                    x2s = small.tile([P, BT], F32)
                    nc.vector.tensor_scalar_mul(x2s, x2cols, 1.0 / NCORES)
                    # bf16 copy of x for the tail's dot product (the dot term
                    # contributes ~1e-5 of the output; bf16 is plenty).
                    xb = small.tile([P, BT, D], BF16)
                    nc.vector.tensor_copy(xb, xt)

            # Tail.  S2 fold on the otherwise-idle ACT engine (in parallel
            # with the DVE fold chain): the ones-matmul already summed over
            # partitions (result replicated), so accumulating sqp's free dim
            # leaves S2/N on every partition directly.
            AF = mybir.ActivationFunctionType
            act_scr = scratch.tile([P, MMF], F32)
            s2n = small.tile([P, 1], F32)
            nc.scalar.activation(
                out=act_scr, in_=sqp, func=AF.Identity, scale=1.0 / N,
                accum_out=s2n,
            )

            # S1 fold: PSUM [128, 4*128] -> SBUF [128, 128] (replicated),
            # then down to bf16 for the dot product.
            s1f = small.tile([P, D], F32)
            nc.vector.tensor_reduce(
                out=s1f,
                in_=s1p.rearrange("p (t d) -> p d t", t=MMF // D),
                axis=AX.X,
                op=OP.add,
            )
            s1fb = small.tile([P, D], BF16)
            nc.vector.tensor_copy(s1fb, s1f)

            # dot_j[p] = x[p*8+j] . S1: one multiply against S1 broadcast
            # across the 8 row-blocks via a stride-0 middle AP dim.
            import concourse.bass as bass
            s1rep = bass.AP(
                tensor=s1fb.tensor, offset=s1fb.offset,
                ap=[list(s1fb.ap[0]), [0, BT], list(s1fb.ap[1])],
            )
            xp = scratch.tile([P, BT, D], BF16)
            nc.vector.tensor_mul(out=xp, in0=xb, in1=s1rep)
            dot8 = small.tile([P, BT], F32)
            nc.vector.tensor_reduce(out=dot8, in_=xp, axis=AX.X, op=OP.add)

            # y = x2/8 + (S2/N - (2/N)*dot)
            t1 = small.tile([P, BT], F32)
            nc.vector.tensor_scalar(
                out=t1, in0=dot8, scalar1=-2.0 / N, scalar2=s2n[:, 0:1],
                op0=OP.mult, op1=OP.add,
            )
            y_all = small.tile([P, BT], F32)
            nc.vector.tensor_add(y_all, t1, x2s)
            nc.sync.dma_start(out=y_d, in_=y_all)
    nc.compile()
    return nc


_nc_cache = None


def _get_nc():
    global _nc_cache
    if _nc_cache is None:
        _nc_cache = _build()
    return _nc_cache


def make_in_maps(x: np.ndarray, features: np.ndarray) -> list[dict[str, np.ndarray]]:
    x = np.ascontiguousarray(x, dtype=np.float32)
    features = np.ascontiguousarray(features, dtype=np.float32)
    padded = np.zeros((PAD_N, D), dtype=np.float32)
    padded[: features.shape[0]] = features
    return [
        {"features": padded[c * RPC : (c + 1) * RPC], "x": x}
        for c in range(NCORES)
    ]


def kernel(x: np.ndarray, features: np.ndarray, _trace: bool = False):
    nc = _get_nc()
    in_maps = make_in_maps(x, features)
    res = bass_utils.run_bass_kernel_spmd(
        nc, in_maps, core_ids=list(range(NCORES)), trace=_trace
    )
    out = np.zeros(B, dtype=np.float64)
    for c in range(NCORES):
        # y[p, t] holds output row p*BT + t, so row-major reshape is exact.
        out += res.results[c]["y"].reshape(B).astype(np.float64)
    out = out.astype(np.float32)
    if _trace:
        return out, res
    return out


# revision 31
# speedup vs baseline: 1.0675x; 1.0675x over previous
"""Distributed mean-squared-distance kernel for Trainium2 (8 NeuronCores).

Computes  out[b] = mean_n ||x[b] - features[n]||^2  for x:[1024,128],
features:[100000,128].

Because the mean is linear, the full [B, N] distance matrix is never needed:

    out[b] = ||x_b||^2 + (1/N) * sum_n ||f_n||^2 - (2/N) * x_b . (sum_n f_n)

Each core streams a 1/8 shard of `features` once (memory-bound roofline:
~6.55 MB/core at ~350 GB/s).  The shard is cast fp32->bf16 inside the DMA
(SWDGE inline cast): HBM traffic is unchanged, but every downstream engine
runs at its 16-bit fast path.  Precision loss is ~1e-5 relative -- the
noise-sensitive |x|^2 term stays fp32.

Engine split, all overlapped with the DMA stream:

  * PE: S1 = sum_n f_n via an all-ones bf16 stationary matrix -- the
    ones-matmul output is the column sum replicated across all 128 output
    partitions, so no cross-partition reduce is ever needed.  26 bf16
    matmuls (free dim 512) accumulate into one PSUM bank.  A final
    ones/N matmul likewise turns the per-partition sum-of-squares column
    into a replicated S2/N scalar.
  * DVE: per-tile square+reduce in bf16 (2 elem/cycle), the fp32 x-path,
    and the final combine.
  * ACT / GPSIMD-compute: unused (per-instruction overhead).

The host gather step sums the 8 partial outputs (the all-reduce of the
sharding hint).
"""

import sys

sys.path.insert(0, "/opt/trn_rl_repo")

import numpy as np

import concourse.bacc as bacc
import concourse.tile as tile
from concourse import mybir
from concourse import bass_utils

P = 128                    # SBUF partitions
B, D, N = 1024, 128, 100000
NCORES = 8
TPP = 100                  # feature rows per partition per core
RPC = P * TPP              # 12800 feature rows per core (padded shard)
PAD_N = RPC * NCORES       # 102400 rows after zero-padding
BT = B // P                # 8 x-rows per partition
# Feature DMA tiles (rows-per-partition each).  The first two go over HWDGE
# as fp32 (the Sync engine is ready ~2us before the SWDGE Q7 path, so this
# fills the otherwise half-idle head of the stream); the rest are SWDGE
# casting DMAs.  The small tile goes last so the post-stream dependency
# chain (last tile's square + matmuls + completion-receipt lag) is short.
HW_CHUNKS = []
SW_CHUNKS = [8] * 12 + [4]
CHUNKS = HW_CHUNKS + SW_CHUNKS
MMF = 512                  # matmul moving free size (one PSUM bank of fp32)

F32 = mybir.dt.float32
BF16 = mybir.dt.bfloat16
AX = mybir.AxisListType
OP = mybir.AluOpType


def _build():
    nc = bacc.Bacc("TRN2", debug=False, num_devices=NCORES)
    f_d = nc.dram_tensor("features", [RPC, D], F32, kind="ExternalInput").ap()
    x_d = nc.dram_tensor("x", [B, D], F32, kind="ExternalInput").ap()
    y_d = nc.dram_tensor("y", [P, BT], F32, kind="ExternalOutput").ap()

    # Row r of the shard maps to partition r // TPP, chunk r % TPP: each
    # partition reads one contiguous (TPP*D*4 B) run of DRAM per core.
    f_view = f_d.rearrange("(p t) d -> p t d", p=P)    # [128, 100, 128]
    x_view = x_d.rearrange("(p t) d -> p t d", p=P)    # [128, 8, 128]

    with tile.TileContext(nc) as tc:
        with (
            tc.tile_pool(name="fpool", bufs=1) as fpool,
            tc.tile_pool(name="scratch", bufs=1) as scratch,
            tc.tile_pool(name="small", bufs=1) as small,
            tc.tile_pool(name="psum", bufs=1, space="PSUM") as psum,
        ):
            # x via HWDGE (no cast; the x-path stays fp32).
            xt = small.tile([P, BT, D], F32)
            nc.sync.dma_start(out=xt, in_=x_view)

            ones = small.tile([P, P], BF16)
            nc.vector.memset(ones, 1.0)
            onesf = small.tile([P, P], F32)
            nc.vector.memset(onesf, 1.0)

            # The PE boots throttled (HAM clock gate, 1.2 GHz) and only
            # reaches 2.4 GHz after ~3.4us of sustained activity.  Keep it
            # busy with junk matmuls through the otherwise-dead window before
            # the first feature tile lands, so the real matmuls run warm.
            warmp = psum.tile([P, P], F32)
            for w in range(80):
                nc.tensor.matmul(
                    warmp, lhsT=ones, rhs=ones, start=True, stop=True,
                    skip_group_check=True,
                )

            # Feature stream: per tile one casting DMA, one DVE bf16 square,
            # and accumulating PE ones-matmuls for BOTH reductions -- S1 from
            # the raw tile, and the squared tile's column sums (toward S2)
            # into a second PSUM bank.  DVE never reduces the stream.
            s1p = psum.tile([P, MMF], F32)
            sqp = psum.tile([P, MMF], F32)
            nmm_total = sum(-(-sz * D // MMF) for sz in CHUNKS)
            sqscr = [
                scratch.tile([P, CHUNKS[0] * D], BF16, name=f"sqscr{k}")
                for k in range(4)
            ]
            sqscrf = [
                scratch.tile([P, CHUNKS[0] * D], F32, name=f"sqscrf{k}")
                for k in range(len(HW_CHUNKS))
            ]
            mm_idx = 0
            off = 0
            for i, sz in enumerate(CHUNKS):
                hw = i < len(HW_CHUNKS)
                dt_t = F32 if hw else BF16
                ft = fpool.tile([P, sz, D], dt_t, tag=f"ft{i}")
                if hw:
                    nc.sync.dma_start(out=ft, in_=f_view[:, off : off + sz, :])
                    scr = sqscrf[i]
                    lhs = onesf
                else:
                    nc.gpsimd.dma_start(
                        out=ft, in_=f_view[:, off : off + sz, :]
                    )
                    scr = sqscr[i % 4]
                    lhs = ones
                flat = ft.rearrange("p t d -> p (t d)")
                nc.vector.tensor_mul(out=scr[:, : sz * D], in0=flat, in1=flat)
                for j0 in range(0, sz * D, MMF):
                    j1 = min(j0 + MMF, sz * D)
                    first = mm_idx == 0
                    last = mm_idx == nmm_total - 1
                    nc.tensor.matmul(
                        s1p[:, : j1 - j0],
                        lhsT=lhs,
                        rhs=flat[:, j0:j1],
                        start=first,
                        stop=last,
                        skip_group_check=True,
                    )
                    nc.tensor.matmul(
                        sqp[:, : j1 - j0],
                        lhsT=lhs,
                        rhs=scr[:, j0:j1],
                        start=first,
                        stop=last,
                        skip_group_check=True,
                    )
                    mm_idx += 1
                off += sz
                if i == 5:
                    # x2 path, emitted mid-stream: x has arrived by now, and
                    # putting it first would park unready instructions at the
                    # DVE queue head (wait-queue depth 4) and stall the tile
                    # squares behind them.
                    xx = scratch.tile([P, BT, D], F32)
                    nc.vector.tensor_mul(out=xx, in0=xt, in1=xt)
                    x2cols = small.tile([P, BT], F32)
                    nc.vector.tensor_reduce(
                        out=x2cols, in_=xx, axis=AX.X, op=OP.add
                    )
                    x2s = small.tile([P, BT], F32)
                    nc.vector.tensor_scalar_mul(x2s, x2cols, 1.0 / NCORES)
                    # bf16 copy of x for the tail's dot product (the dot term
                    # contributes ~1e-5 of the output; bf16 is plenty).
                    xb = small.tile([P, BT, D], BF16)
                    nc.vector.tensor_copy(xb, xt)

            # Tail.  S2 fold on the otherwise-idle ACT engine (in parallel
            # with the DVE fold chain): the ones-matmul already summed over
            # partitions (result replicated), so accumulating sqp's free dim
            # leaves S2/N on every partition directly.
            AF = mybir.ActivationFunctionType
            act_scr = scratch.tile([P, MMF], F32)
            s2n = small.tile([P, 1], F32)
            nc.scalar.activation(
                out=act_scr, in_=sqp, func=AF.Identity, scale=1.0 / N,
                accum_out=s2n,
            )

            # S1 fold: PSUM [128, 4*128] -> SBUF [128, 128] (replicated),
            # then down to bf16 for the dot product.
            s1f = small.tile([P, D], F32)
            nc.vector.tensor_reduce(
                out=s1f,
                in_=s1p.rearrange("p (t d) -> p d t", t=MMF // D),
                axis=AX.X,
                op=OP.add,
            )
            s1fb = small.tile([P, D], BF16)
            nc.vector.tensor_copy(s1fb, s1f)

            # dot_j[p] = x[p*8+j] . S1: one multiply against S1 broadcast
            # across the 8 row-blocks via a stride-0 middle AP dim.
            import concourse.bass as bass
            s1rep = bass.AP(
                tensor=s1fb.tensor, offset=s1fb.offset,
                ap=[list(s1fb.ap[0]), [0, BT], list(s1fb.ap[1])],
            )
            xp = scratch.tile([P, BT, D], BF16)
            nc.vector.tensor_mul(out=xp, in0=xb, in1=s1rep)
            dot8 = small.tile([P, BT], F32)
            nc.vector.tensor_reduce(out=dot8, in_=xp, axis=AX.X, op=OP.add)

            # y = x2/8 + (S2/N - (2/N)*dot)
            t1 = small.tile([P, BT], F32)
            nc.vector.tensor_scalar(
                out=t1, in0=dot8, scalar1=-2.0 / N, scalar2=s2n[:, 0:1],
                op0=OP.mult, op1=OP.add,
            )
            y_all = small.tile([P, BT], F32)
            nc.vector.tensor_add(y_all, t1, x2s)
            nc.sync.dma_start(out=y_d, in_=y_all)
    nc.compile()
    return nc


_nc_cache = None


def _get_nc():
    global _nc_cache
    if _nc_cache is None:
        _nc_cache = _build()
    return _nc_cache


def make_in_maps(x: np.ndarray, features: np.ndarray) -> list[dict[str, np.ndarray]]:
    x = np.ascontiguousarray(x, dtype=np.float32)
    features = np.ascontiguousarray(features, dtype=np.float32)
    padded = np.zeros((PAD_N, D), dtype=np.float32)
    padded[: features.shape[0]] = features
    return [
        {"features": padded[c * RPC : (c + 1) * RPC], "x": x}
        for c in range(NCORES)
    ]


def kernel(x: np.ndarray, features: np.ndarray, _trace: bool = False):
    nc = _get_nc()
    in_maps = make_in_maps(x, features)
    res = bass_utils.run_bass_kernel_spmd(
        nc, in_maps, core_ids=list(range(NCORES)), trace=_trace
    )
    out = np.zeros(B, dtype=np.float64)
    for c in range(NCORES):
        # y[p, t] holds output row p*BT + t, so row-major reshape is exact.
        out += res.results[c]["y"].reshape(B).astype(np.float64)
    out = out.astype(np.float32)
    if _trace:
        return out, res
    return out


# revision 33
# speedup vs baseline: 1.1153x; 1.0448x over previous
"""Distributed mean-squared-distance kernel for Trainium2 (8 NeuronCores).

Computes  out[b] = mean_n ||x[b] - features[n]||^2  for x:[1024,128],
features:[100000,128].

Because the mean is linear, the full [B, N] distance matrix is never needed:

    out[b] = ||x_b||^2 + (1/N) * sum_n ||f_n||^2 - (2/N) * x_b . (sum_n f_n)

Each core streams a 1/8 shard of `features` once (memory-bound roofline:
~6.55 MB/core at ~350 GB/s).  The shard is cast fp32->bf16 inside the DMA
(SWDGE inline cast): HBM traffic is unchanged, but every downstream engine
runs at its 16-bit fast path.  Precision loss is ~1e-5 relative -- the
noise-sensitive |x|^2 term stays fp32.

Engine split, all overlapped with the DMA stream:

  * PE: S1 = sum_n f_n via an all-ones bf16 stationary matrix -- the
    ones-matmul output is the column sum replicated across all 128 output
    partitions, so no cross-partition reduce is ever needed.  26 bf16
    matmuls (free dim 512) accumulate into one PSUM bank.  A final
    ones/N matmul likewise turns the per-partition sum-of-squares column
    into a replicated S2/N scalar.
  * DVE: per-tile square+reduce in bf16 (2 elem/cycle), the fp32 x-path,
    and the final combine.
  * ACT / GPSIMD-compute: unused (per-instruction overhead).

The host gather step sums the 8 partial outputs (the all-reduce of the
sharding hint).
"""

import sys

sys.path.insert(0, "/opt/trn_rl_repo")

import numpy as np

import concourse.bacc as bacc
import concourse.tile as tile
from concourse import mybir
from concourse import bass_utils

P = 128                    # SBUF partitions
B, D, N = 1024, 128, 100000
NCORES = 8
TPP = 100                  # feature rows per partition per core
RPC = P * TPP              # 12800 feature rows per core (padded shard)
PAD_N = RPC * NCORES       # 102400 rows after zero-padding
BT = B // P                # 8 x-rows per partition
# Feature DMA tiles (rows-per-partition each).  The first two go over HWDGE
# as fp32 (the Sync engine is ready ~2us before the SWDGE Q7 path, so this
# fills the otherwise half-idle head of the stream); the rest are SWDGE
# casting DMAs.  The small tile goes last so the post-stream dependency
# chain (last tile's square + matmuls + completion-receipt lag) is short.
HW_CHUNKS = []
SW_CHUNKS = [8] * 12 + [4]
CHUNKS = HW_CHUNKS + SW_CHUNKS
MMF = 512                  # matmul moving free size (one PSUM bank of fp32)

F32 = mybir.dt.float32
BF16 = mybir.dt.bfloat16
AX = mybir.AxisListType
OP = mybir.AluOpType


def _build():
    nc = bacc.Bacc("TRN2", debug=False, num_devices=NCORES)
    f_d = nc.dram_tensor("features", [RPC, D], F32, kind="ExternalInput").ap()
    x_d = nc.dram_tensor("x", [B, D], F32, kind="ExternalInput").ap()
    y_d = nc.dram_tensor("y", [P, BT], F32, kind="ExternalOutput").ap()

    # Row r of the shard maps to partition r // TPP, chunk r % TPP: each
    # partition reads one contiguous (TPP*D*4 B) run of DRAM per core.
    f_view = f_d.rearrange("(p t) d -> p t d", p=P)    # [128, 100, 128]
    x_view = x_d.rearrange("(p t) d -> p t d", p=P)    # [128, 8, 128]

    with tile.TileContext(nc) as tc:
        with (
            tc.tile_pool(name="fpool", bufs=1) as fpool,
            tc.tile_pool(name="scratch", bufs=1) as scratch,
            tc.tile_pool(name="small", bufs=1) as small,
            tc.tile_pool(name="psum", bufs=1, space="PSUM") as psum,
        ):
            # x via HWDGE (no cast; the x-path stays fp32).
            xt = small.tile([P, BT, D], F32)
            nc.sync.dma_start(out=xt, in_=x_view)

            ones = small.tile([P, P], BF16)
            nc.vector.memset(ones, 1.0)
            if HW_CHUNKS:
                onesf = small.tile([P, P], F32)
                nc.vector.memset(onesf, 1.0)

            # The PE boots throttled (HAM clock gate, 1.2 GHz) and only
            # reaches 2.4 GHz after ~3.4us of sustained activity.  Keep it
            # busy with junk matmuls through the otherwise-dead window before
            # the first feature tile lands, so the real matmuls run warm.
            warmp = psum.tile([P, P], F32)
            for w in range(80):
                nc.tensor.matmul(
                    warmp, lhsT=ones, rhs=ones, start=True, stop=True,
                    skip_group_check=True,
                )

            # Feature stream: per tile one casting DMA, one DVE bf16 square,
            # and accumulating PE ones-matmuls for BOTH reductions -- S1 from
            # the raw tile, and the squared tile's column sums (toward S2)
            # into a second PSUM bank.  DVE never reduces the stream.
            s1p = psum.tile([P, MMF], F32)
            sqp = psum.tile([P, MMF], F32)
            nmm_total = sum(-(-sz * D // MMF) for sz in CHUNKS)
            sqscr = [
                scratch.tile([P, CHUNKS[0] * D], BF16, name=f"sqscr{k}")
                for k in range(4)
            ]
            sqscrf = [
                scratch.tile([P, CHUNKS[0] * D], F32, name=f"sqscrf{k}")
                for k in range(len(HW_CHUNKS))
            ]
            mm_idx = 0
            off = 0
            for i, sz in enumerate(CHUNKS):
                hw = i < len(HW_CHUNKS)
                dt_t = F32 if hw else BF16
                ft = fpool.tile([P, sz, D], dt_t, tag=f"ft{i}")
                if hw:
                    nc.sync.dma_start(out=ft, in_=f_view[:, off : off + sz, :])
                    scr = sqscrf[i]
                    lhs = onesf
                else:
                    nc.gpsimd.dma_start(
                        out=ft, in_=f_view[:, off : off + sz, :]
                    )
                    scr = sqscr[i % 4]
                    lhs = ones
                flat = ft.rearrange("p t d -> p (t d)")
                nc.vector.tensor_mul(out=scr[:, : sz * D], in0=flat, in1=flat)
                for j0 in range(0, sz * D, MMF):
                    j1 = min(j0 + MMF, sz * D)
                    first = mm_idx == 0
                    last = mm_idx == nmm_total - 1
                    nc.tensor.matmul(
                        s1p[:, : j1 - j0],
                        lhsT=lhs,
                        rhs=flat[:, j0:j1],
                        start=first,
                        stop=last,
                        skip_group_check=True,
                    )
                    nc.tensor.matmul(
                        sqp[:, : j1 - j0],
                        lhsT=lhs,
                        rhs=scr[:, j0:j1],
                        start=first,
                        stop=last,
                        skip_group_check=True,
                    )
                    mm_idx += 1
                off += sz
                if i == 5:
                    # x2 path, emitted mid-stream: x has arrived by now, and
                    # putting it first would park unready instructions at the
                    # DVE queue head (wait-queue depth 4) and stall the tile
                    # squares behind them.
                    xx = scratch.tile([P, BT, D], F32)
                    nc.vector.tensor_mul(out=xx, in0=xt, in1=xt)
                    x2cols = small.tile([P, BT], F32)
                    nc.vector.tensor_reduce(
                        out=x2cols, in_=xx, axis=AX.X, op=OP.add
                    )
                    x2s = small.tile([P, BT], F32)
                    nc.vector.tensor_scalar_mul(x2s, x2cols, 1.0 / NCORES)
                    # bf16 copy of x for the tail's dot product (the dot term
                    # contributes ~1e-5 of the output; bf16 is plenty).
                    xb = small.tile([P, BT, D], BF16)
                    nc.vector.tensor_copy(xb, xt)

            # Tail.  S2 fold on the otherwise-idle ACT engine (in parallel
            # with the DVE fold chain): the ones-matmul already summed over
            # partitions (result replicated), so accumulating sqp's free dim
            # leaves S2/N on every partition directly.
            AF = mybir.ActivationFunctionType
            act_scr = scratch.tile([P, MMF], F32)
            s2n = small.tile([P, 1], F32)
            nc.scalar.activation(
                out=act_scr, in_=sqp, func=AF.Identity, scale=1.0 / N,
                accum_out=s2n,
            )

            # S1 fold: PSUM [128, 4*128] -> SBUF [128, 128] (replicated),
            # then down to bf16 for the dot product.
            s1fb = small.tile([P, D], BF16)
            with nc.allow_low_precision(reason="S1 fold feeds the bf16 dot"):
                nc.vector.tensor_reduce(
                    out=s1fb,
                    in_=s1p.rearrange("p (t d) -> p d t", t=MMF // D),
                    axis=AX.X,
                    op=OP.add,
                )

            # dot_j[p] = x[p*8+j] . S1: one multiply against S1 broadcast
            # across the 8 row-blocks via a stride-0 middle AP dim.
            import concourse.bass as bass
            s1rep = bass.AP(
                tensor=s1fb.tensor, offset=s1fb.offset,
                ap=[list(s1fb.ap[0]), [0, BT], list(s1fb.ap[1])],
            )
            xp = scratch.tile([P, BT, D], BF16)
            nc.vector.tensor_mul(out=xp, in0=xb, in1=s1rep)
            dot8 = small.tile([P, BT], F32)
            nc.vector.tensor_reduce(out=dot8, in_=xp, axis=AX.X, op=OP.add)

            # y = x2/8 + (S2/N - (2/N)*dot)
            t1 = small.tile([P, BT], F32)
            nc.vector.tensor_scalar(
                out=t1, in0=dot8, scalar1=-2.0 / N, scalar2=s2n[:, 0:1],
                op0=OP.mult, op1=OP.add,
            )
            y_all = small.tile([P, BT], F32)
            nc.vector.tensor_add(y_all, t1, x2s)
            nc.sync.dma_start(out=y_d, in_=y_all)
    nc.compile()
    return nc


_nc_cache = None


def _get_nc():
    global _nc_cache
    if _nc_cache is None:
        _nc_cache = _build()
    return _nc_cache


def make_in_maps(x: np.ndarray, features: np.ndarray) -> list[dict[str, np.ndarray]]:
    x = np.ascontiguousarray(x, dtype=np.float32)
    features = np.ascontiguousarray(features, dtype=np.float32)
    padded = np.zeros((PAD_N, D), dtype=np.float32)
    padded[: features.shape[0]] = features
    return [
        {"features": padded[c * RPC : (c + 1) * RPC], "x": x}
        for c in range(NCORES)
    ]


def kernel(x: np.ndarray, features: np.ndarray, _trace: bool = False):
    nc = _get_nc()
    in_maps = make_in_maps(x, features)
    res = bass_utils.run_bass_kernel_spmd(
        nc, in_maps, core_ids=list(range(NCORES)), trace=_trace
    )
    out = np.zeros(B, dtype=np.float64)
    for c in range(NCORES):
        # y[p, t] holds output row p*BT + t, so row-major reshape is exact.
        out += res.results[c]["y"].reshape(B).astype(np.float64)
    out = out.astype(np.float32)
    if _trace:
        return out, res
    return out
